# revision 11
# baseline (speedup 1.0000x reference)
"""GQA attention kernel for 8 Trainium2 NeuronCores.

Sharding: core i handles batch b=i//2 and query-token half th=i%2 (data
parallel over batch x query split). Each core computes q projection for its
256 tokens (all 16 heads), duplicates the cheap k/v projection for its batch,
runs full attention against the 4096-entry KV (cache + new), and the complete
o-projection for its token half. No collectives; unshard is concatenation.

Host-side prep is layout only: per-core roll of x so every core's q tokens are
rows 0:256 (one SPMD program works for all cores), transposes, bf16 casts, and
RoPE table gathers with the (identity) per-head norm weights folded in.

Device layouts: scores are computed transposed [s, t] so exp(scores) feeds the
P@V matmul directly with v in natural [s, d] layout; softmax denominators come
from an all-ones stationary matmul (fp32 PSUM accumulate, broadcast over
partitions); normalization is fused into the ctx PSUM drain.
"""

import numpy as np
import ml_dtypes

B, T, D, H, G, HD, CACHE = 4, 512, 2048, 16, 4, 128, 3584
S = CACHE + T
TH = T // 2
KO = D // 128
SJ = S // 128
NCORES = 8
EPS = 1e-6
BF = ml_dtypes.bfloat16

_prog_cache = {}


def _build(masked):
    import concourse.mybir as mybir
    import concourse.tile as tile
    from concourse import bacc
    from concourse.bass import ts, ds
    from contextlib import ExitStack

    bf = mybir.dt.bfloat16
    f32 = mybir.dt.float32
    AF = mybir.ActivationFunctionType
    AL = mybir.AluOpType
    AX = mybir.AxisListType

    nc = bacc.Bacc(None, target_bir_lowering=False)
    xT = nc.dram_tensor("xT", [D, T], bf, kind="ExternalInput")
    qw = nc.dram_tensor("qw", [D, H * HD], bf, kind="ExternalInput")
    kw = nc.dram_tensor("kw", [D, G * HD], bf, kind="ExternalInput")
    vw = nc.dram_tensor("vw", [D, G * HD], bf, kind="ExternalInput")
    ow = nc.dram_tensor("ow", [H * HD, D], bf, kind="ExternalInput")
    ckT = nc.dram_tensor("ckT", [G, HD, CACHE], bf, kind="ExternalInput")
    cvr = nc.dram_tensor("cvr", [G, CACHE, HD], bf, kind="ExternalInput")
    cq = nc.dram_tensor("cq", [TH, HD], f32, kind="ExternalInput")
    sq = nc.dram_tensor("sq", [TH, HD], f32, kind="ExternalInput")
    ck = nc.dram_tensor("ck", [T, HD], f32, kind="ExternalInput")
    sk = nc.dram_tensor("sk", [T, HD], f32, kind="ExternalInput")
    ones_in = nc.dram_tensor("ones_in", [128, 128], bf, kind="ExternalInput")
    if masked:
        keepT = nc.dram_tensor("keepT", [S, TH], bf, kind="ExternalInput")
    outh = nc.dram_tensor("outh", [TH, D], f32, kind="ExternalOutput")
    import os
    _dbg = bool(os.environ.get("KDBG"))
    if _dbg:
        qtd = nc.dram_tensor("qtd", [128, H, TH], bf, kind="ExternalOutput")
        kald = nc.dram_tensor("kald", [128, G, S], bf, kind="ExternalOutput")
        ctd = nc.dram_tensor("ctd", [128, H, TH], bf, kind="ExternalOutput")
        vald = nc.dram_tensor("vald", [128, G, SJ, HD], bf, kind="ExternalOutput")
        scd = nc.dram_tensor("scd", [128, 1024], f32, kind="ExternalOutput")
        ptd = nc.dram_tensor("ptd", [128, 1024], bf, kind="ExternalOutput")
        sumd = nc.dram_tensor("sumd", [4, 128, TH], f32, kind="ExternalOutput")
    knew = nc.dram_tensor("knew", [G, T, HD], f32, kind="ExternalOutput")
    vnew = nc.dram_tensor("vnew", [G, T, HD], f32, kind="ExternalOutput")

    with tile.TileContext(nc) as tc, ExitStack() as ctx:
        sb = ctx.enter_context(tc.tile_pool(name="sb", bufs=1))
        ring = ctx.enter_context(tc.tile_pool(name="ring", bufs=17))
        owring = ctx.enter_context(tc.tile_pool(name="owring", bufs=16))
        ep = ctx.enter_context(tc.tile_pool(name="ep", bufs=1))
        ep2 = ctx.enter_context(tc.tile_pool(name="ep2", bufs=2))
        ptp = ctx.enter_context(tc.tile_pool(name="ptp", bufs=3))
        dram = ctx.enter_context(tc.tile_pool(name="dram", bufs=1, space="DRAM"))
        ps = ctx.enter_context(tc.tile_pool(name="ps", bufs=2, space="PSUM"))
        psacc = ctx.enter_context(tc.tile_pool(name="psacc", bufs=1, space="PSUM"))

        xT_sb = sb.tile([128, KO, T], bf, tag="xT")
        kall = sb.tile([128, G, S], bf, tag="kall")
        vall = sb.tile([128, G, SJ, HD], bf, tag="vall")
        qT_sb = sb.tile([128, H, TH], bf, tag="qT")
        ctxT = sb.tile([128, H, TH], bf, tag="ctxT")
        ones = sb.tile([128, 128], bf, tag="ones")
        cq_sb = sb.tile([128, TH // 128, HD], f32, tag="cq")
        sq_sb = sb.tile([128, TH // 128, HD], f32, tag="sq")
        ck_sb = sb.tile([128, T // 128, HD], f32, tag="ck")
        sk_sb = sb.tile([128, T // 128, HD], f32, tag="sk")

        nc.sync.dma_start(xT_sb[:], xT.rearrange("(ko p) t -> p ko t", p=128))
        for g in range(G):
            nc.sync.dma_start(kall[:, g, :CACHE], ckT[g])
            nc.sync.dma_start(
                vall[:, g, : CACHE // 128, :],
                cvr[g].rearrange("(j p) d -> p j d", p=128),
            )
        nc.sync.dma_start(ones[:], ones_in[:])
        nc.sync.dma_start(cq_sb[:], cq.rearrange("(tt p) d -> p tt d", p=128))
        nc.sync.dma_start(sq_sb[:], sq.rearrange("(tt p) d -> p tt d", p=128))
        nc.sync.dma_start(ck_sb[:], ck.rearrange("(tt p) d -> p tt d", p=128))
        nc.sync.dma_start(sk_sb[:], sk.rearrange("(tt p) d -> p tt d", p=128))
        if masked:
            keep_sb = sb.tile([128, SJ, TH], bf, tag="keep")
            nc.sync.dma_start(
                keep_sb[:], keepT.rearrange("(j p) t -> p j t", p=128)
            )

        def rms_rope(qs, nh, c_ap, s_ap, out_ap):
            # qs: [128, nh, HD] f32 SBUF (modified in place); out_ap same shape.
            sqr = ep.tile([128, nh, HD], f32, tag="m1")
            nc.vector.tensor_tensor(sqr[:], qs, qs, AL.mult)
            ssq = ep2.tile([128, nh], f32, tag="ssq")
            nc.vector.tensor_reduce(ssq[:], sqr[:], AX.X, AL.add)
            nc.vector.tensor_scalar(ssq[:], ssq[:], 1.0 / HD, EPS, AL.mult, AL.add)
            srt = ep2.tile([128, nh], f32, tag="srt")
            nc.scalar.activation(srt[:], ssq[:], AF.Sqrt)
            rin = ep2.tile([128, nh], f32, tag="rin")
            scr = ep2.tile([128, nh], f32, tag="scr")
            nc.vector.reciprocal_approx_accurate(out=rin[:], in_=srt[:], scratch=scr[:])
            nc.vector.tensor_tensor(
                qs, qs, rin[:, :, None].to_broadcast((128, nh, HD)), AL.mult
            )
            m1 = ep.tile([128, nh, HD], f32, tag="m1")
            nc.vector.tensor_tensor(
                m1[:], qs, c_ap[:, None, :].to_broadcast((128, nh, HD)), AL.mult
            )
            r1 = ep.tile([128, nh, 64], f32, tag="r1")
            nc.vector.tensor_tensor(
                r1[:],
                qs[:, :, 64:],
                s_ap[:, None, 0:64].to_broadcast((128, nh, 64)),
                AL.mult,
            )
            nc.vector.tensor_tensor(out_ap[:, :, 0:64], m1[:, :, 0:64], r1[:], AL.subtract)
            r2 = ep.tile([128, nh, 64], f32, tag="r1")
            nc.vector.tensor_tensor(
                r2[:],
                qs[:, :, 0:64],
                s_ap[:, None, 64:].to_broadcast((128, nh, 64)),
                AL.mult,
            )
            nc.vector.tensor_tensor(out_ap[:, :, 64:], m1[:, :, 64:], r2[:], AL.add)

        # ---- q projection (t-half only) + epilogue ----
        qro_dram = dram.tile([TH, H * HD], bf, tag="qrd")
        q_stage = []
        for tt in range(2):
            q_stage.append(ep.tile([128, H, HD], f32, tag=f"qs{tt}", name=f"qs{tt}"))
        for nn in range(4):
            qwt = []
            for ko in range(KO):
                t_ = ring.tile([128, 512], bf, tag="qw")
                nc.sync.dma_start(t_[:], qw[ds(ko * 128, 128), ds(nn * 512, 512)])
                qwt.append(t_)
            for tt in range(2):
                pq = ps.tile([128, 1024], f32, tag="ps")
                for ko in range(KO):
                    nc.tensor.matmul(
                        pq[:, :512],
                        xT_sb[:, ko, ts(tt, 128)],
                        qwt[ko][:],
                        start=(ko == 0),
                        stop=(ko == KO - 1),
                    )
                nc.scalar.activation(q_stage[tt][:, ts(nn, 4), :], pq[:, :512], AF.Copy)
        for tt in range(2):
            qrb = ep.tile([128, H, HD], bf, tag="qrb")
            rms_rope(q_stage[tt][:], H, cq_sb[:, tt, :], sq_sb[:, tt, :], qrb)
            nc.sync.dma_start(
                qro_dram[ts(tt, 128), :], qrb[:].rearrange("p h d -> p (h d)")
            )
        for h in range(H):
            nc.sync.dma_start_transpose(qT_sb[:, h, :], qro_dram[:, ds(h * 128, 128)])

        # ---- k/v projection (full T) + epilogue ----
        kro_dram = dram.tile([T, G * HD], bf, tag="krd")
        kwt = []
        for ko in range(KO):
            t_ = ring.tile([128, 512], bf, tag="qw", name=f"kw{ko}")
            nc.sync.dma_start(t_[:], kw[ds(ko * 128, 128), :])
            kwt.append(t_)
        for tt in range(4):
            pk = ps.tile([128, 1024], f32, tag="ps")
            for ko in range(KO):
                nc.tensor.matmul(
                    pk[:, :512],
                    xT_sb[:, ko, ts(tt, 128)],
                    kwt[ko][:],
                    start=(ko == 0),
                    stop=(ko == KO - 1),
                )
            k_st = ep2.tile([128, G, HD], f32, tag="kst")
            nc.scalar.activation(k_st[:], pk[:, :512], AF.Copy)
            k_ro = ep2.tile([128, G, HD], f32, tag="kro")
            rms_rope(k_st[:], G, ck_sb[:, tt, :], sk_sb[:, tt, :], k_ro)
            nc.sync.dma_start(
                knew[:, ts(tt, 128), :].transpose([1, 0, 2]),
                k_ro[:],
            )
            k_rb = ep2.tile([128, G * HD], bf, tag="krb")
            nc.vector.tensor_copy(out=k_rb[:], in_=k_ro[:].rearrange("p g d -> p (g d)"))
            nc.sync.dma_start(kro_dram[ts(tt, 128), :], k_rb[:])

        vwt = []
        for ko in range(KO):
            t_ = ring.tile([128, 512], bf, tag="qw", name=f"vw{ko}")
            nc.sync.dma_start(t_[:], vw[ds(ko * 128, 128), :])
            vwt.append(t_)
        for tt in range(4):
            pv = ps.tile([128, 1024], f32, tag="ps")
            for ko in range(KO):
                nc.tensor.matmul(
                    pv[:, :512],
                    xT_sb[:, ko, ts(tt, 128)],
                    vwt[ko][:],
                    start=(ko == 0),
                    stop=(ko == KO - 1),
                )
            v_st = ep2.tile([128, G, HD], f32, tag="vst")
            nc.scalar.activation(v_st[:], pv[:, :512], AF.Copy)
            nc.sync.dma_start(
                vnew[:, ts(tt, 128), :].transpose([1, 0, 2]),
                v_st[:],
            )
            nc.vector.tensor_copy(out=vall[:, :, CACHE // 128 + tt, :], in_=v_st[:])
        for g in range(G):
            nc.sync.dma_start_transpose(
                kall[:, g, CACHE:], kro_dram[:, ds(g * 128, 128)]
            )

        # ---- attention ----
        inv_sqrt_hd = float(1.0 / np.sqrt(HD))
        for g in range(G):
            # one PSUM bank per head: cols [h4*512, h4*512+256) = ctx accum,
            # cols [h4*512+256, (h4+1)*512) = softmax denominator accum.
            # start=True clears has_written for the WHOLE bank, so the two
            # accumulation chains sharing a bank must start with exactly one
            # start=True (ctx, emitted first) and an overwrite-on-unset
            # start=False first write for the sums chain.
            ctxsum = psacc.tile([128, 2048], f32, tag="cs")
            for j in range(SJ):
                sc = ps.tile([128, 1024], f32, tag="ps")
                for h4 in range(4):
                    nc.tensor.matmul(
                        sc[:, ts(h4, TH)],
                        kall[:, g, ts(j, 128)],
                        qT_sb[:, g * 4 + h4, :],
                        start=True,
                        stop=True,
                    )
                pt = ptp.tile([128, 4, TH], bf, tag="pt")
                if _dbg and g == 0 and j == 0:
                    scdbg = ep2.tile([128, 1024], f32, tag="scdbg", name="scdbg")
                    nc.scalar.activation(scdbg[:], sc[:], AF.Copy)
                    nc.sync.dma_start(scd[:], scdbg[:])
                nc.scalar.activation(
                    pt[:].rearrange("p h t -> p (h t)"), sc[:], AF.Exp, scale=inv_sqrt_hd
                )
                if _dbg and g == 0 and j == 0:
                    nc.sync.dma_start(ptd[:], pt[:].rearrange("p h t -> p (h t)"))
                if masked:
                    nc.vector.tensor_tensor(
                        pt[:],
                        pt[:],
                        keep_sb[:, j, None, :].to_broadcast((128, 4, TH)),
                        AL.mult,
                    )
                for h4 in range(4):
                    nc.tensor.matmul(
                        ctxsum[:, ds(h4 * 512, TH)],
                        vall[:, g, j, :],
                        pt[:, h4, :],
                        start=(j == 0),
                        stop=(j == SJ - 1),
                    )
                    nc.tensor.matmul(
                        ctxsum[:, ds(h4 * 512 + 256, TH)],
                        ones[:],
                        pt[:, h4, :],
                        start=False,
                        stop=(j == SJ - 1),
                        skip_group_check=True,
                    )
            for h4 in range(4):
                sums_sb = ep2.tile([128, TH], f32, tag="ssb")
                nc.scalar.activation(sums_sb[:], ctxsum[:, ds(h4 * 512 + 256, TH)], AF.Copy)
                if _dbg and g == 0:
                    nc.sync.dma_start(sumd[h4], sums_sb[:])
                rin = ep2.tile([128, TH], f32, tag="rinv")
                scr = ep2.tile([128, TH], f32, tag="rscr")
                nc.vector.reciprocal_approx_accurate(
                    out=rin[:], in_=sums_sb[:], scratch=scr[:]
                )
                nc.vector.tensor_tensor(
                    ctxT[:, g * 4 + h4, :], ctxsum[:, ds(h4 * 512, TH)], rin[:], AL.mult
                )

        if _dbg:
            nc.sync.dma_start(qtd[:], qT_sb[:])
            nc.sync.dma_start(kald[:], kall[:])
            nc.sync.dma_start(ctd[:], ctxT[:])
            nc.sync.dma_start(vald[:], vall[:])

        # ---- o projection ----
        for nn in range(4):
            owt = []
            for h in range(H):
                t_ = owring.tile([128, 512], bf, tag="ow")
                nc.sync.dma_start(t_[:], ow[ds(h * 128, 128), ds(nn * 512, 512)])
                owt.append(t_)
            for tt in range(2):
                po = ps.tile([128, 1024], f32, tag="ps")
                for h in range(H):
                    nc.tensor.matmul(
                        po[:, :512],
                        ctxT[:, h, ts(tt, 128)],
                        owt[h][:],
                        start=(h == 0),
                        stop=(h == H - 1),
                    )
                ost = ep2.tile([128, 512], f32, tag="ost")
                nc.scalar.activation(ost[:], po[:, :512], AF.Copy)
                nc.sync.dma_start(outh[ts(tt, 128), ds(nn * 512, 512)], ost[:])

    nc.compile()
    return nc


def _get_prog(masked):
    if masked not in _prog_cache:
        _prog_cache[masked] = _build(masked)
    return _prog_cache[masked]


def _make_in_maps(inputs):
    x = np.asarray(inputs["x"], np.float32)
    mask = np.asarray(inputs["mask"])
    cos = np.asarray(inputs["cos"], np.float32)
    sin = np.asarray(inputs["sin"], np.float32)
    pids = np.asarray(inputs["position_ids"]).astype(np.int64)
    cache_k = np.asarray(inputs["cache_k"], np.float32)
    cache_v = np.asarray(inputs["cache_v"], np.float32)
    qn_w = np.asarray(inputs["qn_w"], np.float32)
    kn_w = np.asarray(inputs["kn_w"], np.float32)
    masked = bool(mask.any())
    qn_rot = np.roll(qn_w, -64)
    kn_rot = np.roll(kn_w, -64)
    qw_bf = np.asarray(inputs["q_w"], np.float32).astype(BF)
    kw_bf = np.asarray(inputs["k_w"], np.float32).astype(BF)
    vw_bf = np.asarray(inputs["v_w"], np.float32).astype(BF)
    ow_bf = np.asarray(inputs["o_w"], np.float32).astype(BF)
    in_maps = []
    for c in range(NCORES):
        b, th = c // 2, c % 2
        xr = np.roll(x[b], -TH * th, axis=0)
        cos_r = np.roll(cos[pids[b]], -TH * th, axis=0)
        sin_r = np.roll(sin[pids[b]], -TH * th, axis=0)
        m = dict(
            xT=np.ascontiguousarray(xr.T).astype(BF),
            qw=qw_bf,
            kw=kw_bf,
            vw=vw_bf,
            ow=ow_bf,
            ckT=np.ascontiguousarray(cache_k[b].transpose(0, 2, 1)).astype(BF),
            ones_in=np.ones((128, 128), BF),
            cvr=cache_v[b].astype(BF),
            cq=np.ascontiguousarray(cos_r[:TH] * qn_w),
            sq=np.ascontiguousarray(sin_r[:TH] * qn_rot),
            ck=np.ascontiguousarray(cos_r * kn_w),
            sk=np.ascontiguousarray(sin_r * kn_rot),
        )
        if masked:
            keep = (~mask[b, 0]).astype(np.float32).T  # [S, T]
            keep = np.concatenate(
                [keep[:CACHE], np.roll(keep[CACHE:], -TH * th, axis=0)], axis=0
            )
            keep = np.roll(keep, -TH * th, axis=1)[:, :TH]
            m["keepT"] = np.ascontiguousarray(keep).astype(BF)
        in_maps.append(m)
    return in_maps, masked, cache_k, cache_v


def _gather(results, cache_k, cache_v):
    out = np.empty((B, T, D), np.float32)
    k_full = np.empty((B, G, S, HD), np.float32)
    v_full = np.empty((B, G, S, HD), np.float32)
    for b in range(B):
        k_full[b, :, :CACHE] = cache_k[b]
        v_full[b, :, :CACHE] = cache_v[b]
        r0 = results[2 * b]
        r1 = results[2 * b + 1]
        out[b, :TH] = r0["outh"]
        out[b, TH:] = r1["outh"]
        k_full[b, :, CACHE:] = r0["knew"]
        v_full[b, :, CACHE:] = r0["vnew"]
    return out, k_full, v_full


def run_spmd(inputs, trace=False, **kw):
    from concourse.bass_utils import run_bass_kernel_spmd

    in_maps, masked, cache_k, cache_v = _make_in_maps(inputs)
    nc = _get_prog(masked)
    res = run_bass_kernel_spmd(
        nc, in_maps, core_ids=list(range(NCORES)), trace=trace, **kw
    )
    return res, cache_k, cache_v


def kernel(**inputs):
    res, cache_k, cache_v = run_spmd(inputs)
    return _gather(res.results, cache_k, cache_v)
